# revision 52
# baseline (speedup 1.0000x reference)
"""LorentzNodeBlock - Trainium2 Bass kernel, 8 NeuronCores, scatter-free.

Sharding: by DESTINATION node (no collective needed). Host sorts nodes by
in-degree into blocks of 72 (one node per (core, group) lane; 8 cores x 9
groups), pads each block's edge list to a common multiple-of-4 degree, and
packs blocks into 6144-slot "supers" so every lane shares one static layout.

Per-edge pipeline on each core (T ~ 92k slots, 126 partitions = 9 groups x
14 features):
  * q = minkowski(x)[row] is folded into edge_attr on the host
    (attr~ = attr + alpha q, alpha = W_attr^-T w_q, exact) so the payload
    is just 14 fp8-e4m3 rows per group -> 12 MB/core of HBM traffic.
  * PE: block-diagonal first-layer matmul (bf16 weights x fp8 payload)
    into PSUM f32, 512-col matmuls, 3 per 1536-slot chunk.
  * relu(z + b1a): split between ScalarE activation-with-bias (6/7 of
    chunks, 0.83 ns/el) and DVE tensor_scalar add+max (1/7, 1.04 ns/el)
    to balance the two engines; output bf16 into a per-super tile. The
    ACT table set is warmed by a dummy relu during the initial DMA wait.
  * segment-sum: A+B pairwise folds on DVE at the 2x packed rate
    (0.57 ns/el): first fold the excess over the largest power of two A
    (offset A, width d-A, both mult-4 -> 2x engages), then halve A down
    to 8 with compacting ping-pong writes so every level and the final
    8-wide reduce_sum tail read fully contiguous data. Tails of all runs
    land block-indexed in one region -> one merged reduce per super.
    (InstTensorReduce is capped at 1x; in-place strided tails measured
    2.1 ns/el - both are why the fold tree + compaction exist.)
  * pad slots contribute relu(b1a), subtracted exactly in the node phase
    via an extra K-row of the node matmul (rhs row = n_pad * invdeg).
  * node phase (fused second edge-MLP layer + node MLP, WC = w1b@w2a[1:])
    is emitted piecewise right after the super that completes each
    256-node output range; early pieces ship via the GpSimd SWDGE queue,
    the last ones via sync HWDGE (payload stream is drained by then).
  * aux constants ride the SWDGE queue to stay out of the payload FIFO
    (except the 350KB invdeg table - on SWDGE it starves the output
    pieces - which rides sync after the payload prefetches); the first
    super lands as two chunk-aligned pieces, 4 supers prefetched ahead.
    invdeg/Ssum/outputs are bf16 (b2b bias applied on the host).

Measured on trn2 (8 cores, NTFF): 116.3-117.5 us HW exec (runs vary
~+-1us; occasional device thermal throttling adds 10-20us), rel err
4.4e-3 (vs 136.5 us for the v1 baseline). Engine balance at the wall:
ScalarE ~84us, VectorE ~82us busy; span overhead is ~7us framework
preamble, ~6us DMA ramp, ~5us drain, ~19us chunk-semaphore latency.
"""

import os
import sys
import types
import numpy as np

N = 100000
E = 6400000
HID = 14
NCORES = 8
NGROUPS = 9                   # groups per core
NLANES = NCORES * NGROUPS     # 72 lanes; block = NLANES nodes
ROWS = NGROUPS * HID          # 126 payload/psum partitions
CHUNK = 1536                  # edge slots per PSUM chunk (free dim, 3 banks)
SUPER = 6144                  # edge slots per DMA transfer / fold region
MM_F = 512                    # matmul moving free dim (ISA max; 1 PSUM bank)
NF = 256                      # node-chunk output slot width
DPAD = 4                      # block degree padded to multiple of this
                              # (A+B folds only need mult-4 alignment)
TAIL_STOP = 8                 # stop folding when seg len <= this (or odd)

# engine-balance knobs (per-super assignment)
DVE_RELU_EVERY = 6            # every k-th chunk relu runs on DVE (not ACT)
POOL_FOLD_EVERY = 10 ** 9     # gpsimd folds disabled (2.4 ns/elem measured)

_MINK = np.array([-1.0, 1.0, 1.0, 1.0], dtype=np.float32)


# ---------------------------------------------------------------------------
# axon NTFF shim: lets run_bass_kernel_spmd(trace=True) work when the image's
# antenv package lacks axon_hooks. Harmless when tracing is off.
# ---------------------------------------------------------------------------
def _install_ntff_shim():
    try:
        from antenv.axon_hooks import get_axon_ntff_profile_hook  # noqa: F401
        have = True
    except ImportError:
        have = False
    if not have:
        try:
            import antenv
        except ImportError:
            return
        mod = types.ModuleType("antenv.axon_hooks")
        _hook = [None]
        mod.set_axon_ntff_profile_hook = lambda h: _hook.__setitem__(0, h)
        mod.get_axon_ntff_profile_hook = lambda: _hook[0]
        sys.modules["antenv.axon_hooks"] = mod
        antenv.axon_hooks = mod
    try:
        from antenv.axon_hooks import (get_axon_ntff_profile_hook,
                                       set_axon_ntff_profile_hook)
        if get_axon_ntff_profile_hook() is None:
            from trn_agent_boot.trn_boot import _ntff_profile_via_ctypes
            set_axon_ntff_profile_hook(
                _ntff_profile_via_ctypes('/opt/axon/libaxon_pjrt.so'))
    except Exception:
        pass


# ---------------------------------------------------------------------------
# Host-side layout
# ---------------------------------------------------------------------------
class Layout:
    __slots__ = ("node_sorted", "blk_deg", "blk_off", "n_blocks", "S", "T",
                 "runs", "deg", "cs", "edge_sorted", "n_supers", "Np")


def build_layout(col):
    """col: int array [E] of destination nodes. Pure index work.

    Blocks of NLANES nodes sorted by degree; block degree padded to a
    multiple of DPAD. Blocks packed sequentially into SUPER-slot regions
    (a block never crosses a super boundary). Produces `runs`: maximal
    groups of consecutive equal-d blocks within one super:
        (super_idx, slot_off_in_super, block_base, nb, d)
    """
    lay = Layout()
    deg = np.bincount(col, minlength=N).astype(np.int64)
    Np = ((N + NLANES - 1) // NLANES) * NLANES
    n_pad_nodes = Np - N
    deg_p = np.concatenate([deg, np.zeros(n_pad_nodes, np.int64)])
    node_sorted = np.argsort(deg_p, kind="stable")
    B = Np // NLANES
    node_mat = node_sorted.reshape(B, NLANES)
    raw_deg = deg_p[node_mat].max(axis=1)                     # [B]
    blk_deg = ((raw_deg + DPAD - 1) // DPAD) * DPAD           # mult of DPAD
    assert (blk_deg > 0).all()

    # sequential pack into supers
    blk_off = np.zeros(B, np.int64)                            # global slot
    off = 0
    for b in range(B):
        d = int(blk_deg[b])
        rem = SUPER - (off % SUPER)
        if rem < d:
            off += rem                                        # dead tail
        blk_off[b] = off
        off += d
    T = ((off + CHUNK - 1) // CHUNK) * CHUNK
    n_supers = (T + SUPER - 1) // SUPER

    # runs of equal-d consecutive blocks within one super
    runs = []
    b = 0
    while b < B:
        d = int(blk_deg[b])
        s0 = int(blk_off[b]) // SUPER
        nb = 1
        while (b + nb < B and int(blk_deg[b + nb]) == d
               and int(blk_off[b + nb]) // SUPER == s0
               and int(blk_off[b + nb]) == int(blk_off[b]) + nb * d):
            nb += 1
        runs.append((s0, int(blk_off[b]) % SUPER, b, nb, d))
        b += nb

    cs = np.zeros(N + 1, np.int64)
    cs[1:] = np.cumsum(deg)
    lay.node_sorted = node_sorted
    lay.blk_deg = blk_deg
    lay.blk_off = blk_off
    lay.n_blocks = B
    lay.S = B
    lay.T = int(T)
    lay.n_supers = int(n_supers)
    lay.runs = runs
    lay.deg = deg_p
    lay.cs = cs
    lay.edge_sorted = np.argsort(col, kind="stable")
    lay.Np = Np
    return lay


def build_src_slots(lay):
    """[NLANES, T] int64 edge-id per lane slot, -1 for padding."""
    B, T = lay.n_blocks, lay.T
    node_mat = lay.node_sorted.reshape(B, NLANES)
    deg_nm = lay.deg[node_mat]                       # [B, NLANES]
    D = int(lay.blk_deg.max())
    k = np.arange(D, dtype=np.int64)
    valid = k[None, None, :] < deg_nm[:, :, None]    # [B, NLANES, D]
    cs_nm = np.where(node_mat < N, lay.cs[np.minimum(node_mat, N - 1)], 0)
    idx = cs_nm[:, :, None] + k[None, None, :]
    idx = np.minimum(idx, E - 1)
    src = np.where(valid, lay.edge_sorted[idx], -1)  # [B, NLANES, D]
    slot_valid = k[None, :] < lay.blk_deg[:, None]   # [B, D]
    pos = (lay.blk_off[:, None] + k[None, :])        # [B, D]
    out = np.full((NLANES, T), -1, np.int64)
    out[:, pos[slot_valid]] = src.transpose(1, 0, 2)[:, slot_valid]
    return out


def lane_of(c, g):
    return c * NGROUPS + g


def build_core_payload(lay, src_slots, attr_mod, core, dtype=np.float32):
    """[ROWS, T]: rows 14g+j = attr_mod[:, j] of lane (core, g); pad -> 0."""
    T = lay.T
    pay = np.zeros((ROWS, T), dtype)
    for g in range(NGROUPS):
        s = src_slots[lane_of(core, g)]
        m = s >= 0
        sc = np.where(m, s, 0)
        ea = attr_mod[sc].astype(dtype)
        ea[~m] = 0
        pay[HID * g:HID * (g + 1), :] = ea.T
    return pay


def build_core_aux(lay, x, core):
    """invdeg_rep [ROWS,S], qn [NGROUPS,S], padinv [NGROUPS,S]."""
    B = lay.n_blocks
    node_mat = lay.node_sorted.reshape(B, NLANES)
    lanes = [lane_of(core, g) for g in range(NGROUPS)]
    nodes = node_mat[:, lanes].T                          # [NGROUPS, B]
    degs = lay.deg[nodes].astype(np.float32)              # real degree
    invdeg = 1.0 / np.maximum(degs, 1.0)
    invdeg_rep = np.repeat(invdeg, HID, axis=0)           # [ROWS, B]
    npad = lay.blk_deg[None, :].astype(np.float32) - degs  # padded - real
    padinv = npad * invdeg                                 # [NGROUPS, B]
    real = nodes < N
    xn = x[np.minimum(nodes, N - 1)].astype(np.float32)    # [NGROUPS, B, 4]
    xn[~real] = 0
    qn = ((xn * _MINK) * xn).sum(axis=2).astype(np.float32)  # [NGROUPS, B]
    return invdeg_rep, qn, padinv


def build_weights(w1a, b1a, w1b, b1b, w2a, b2a, w2b, b2b):
    def blkdiag(w, nin, nout):
        out = np.zeros((NGROUPS * nin, NGROUPS * nout), np.float32)
        for g in range(NGROUPS):
            out[g * nin:(g + 1) * nin, g * nout:(g + 1) * nout] = w
        return out
    W_attr = w1a[1:]                                       # [14, 14]
    alpha = np.linalg.solve(W_attr.T.astype(np.float64),
                            w1a[0].astype(np.float64)).astype(np.float32)
    relu_b1a = np.maximum(b1a, 0.0)                        # [14]
    WC = w1b @ w2a[1:]                                     # [14, 14]
    W = {}
    W["alpha"] = alpha
    W["W1blk"] = blkdiag(W_attr, HID, HID)                 # [126, 126]
    W["b1a_rep"] = np.tile(b1a, NGROUPS).astype(np.float32)[:, None]
    W["WCblk"] = blkdiag(WC, HID, HID)
    W["bc_rep"] = np.tile(w2a[1:].T @ b1b + b2a,
                          NGROUPS).astype(np.float32)[:, None]
    W["W2Bblk"] = blkdiag(w2b, HID, HID)
    W["b2b_rep"] = np.tile(b2b, NGROUPS).astype(np.float32)[:, None]
    # node-phase lhs [2*NGROUPS, ROWS]: rows 0..G-1 carry w2a[0] (q term),
    # rows G..2G-1 carry -(WC^T relu_b1a) (pad-slot correction)
    corr = -(WC.T @ relu_b1a)                              # [14]
    lhs = np.zeros((2 * NGROUPS, ROWS), np.float32)
    for g in range(NGROUPS):
        lhs[g, g * HID:(g + 1) * HID] = w2a[0]
        lhs[NGROUPS + g, g * HID:(g + 1) * HID] = corr
    W["NLHS"] = lhs
    return W


def fold_schedule(d):
    """A+B fold plan: ((A, B) | None, [h1, h2, ...], tail).

    HW measurement: the DVE 2x packed mode needs the in1 element offset
    and width to be multiples of 4 (offset%4==2 runs 1x, odd offsets hit
    a ~7x slow path). Plan: first fold the excess over the largest
    power of two A (in1 offset = A, width B = d-A, both mult-8), then
    halve A cleanly down to 8 (offsets = powers of two). Every op is 2x.
    """
    A = 1 << (d.bit_length() - 1)
    if A == d:
        ab = None
    else:
        ab = (A, d - A)
    chain = []
    L = A
    while L > TAIL_STOP:
        L //= 2
        chain.append(L)
    return ab, chain, L


# ---------------------------------------------------------------------------
# Numpy emulation of the device program (for layout/logic/precision check)
# ---------------------------------------------------------------------------
def emulate_core(lay, pay_bf, invdeg_rep, qn, padinv, W, dtype_np):
    import ml_dtypes  # noqa: F401
    bf = dtype_np
    T, S = lay.T, lay.S
    relu = lambda v: np.maximum(v, 0.0)
    W1 = W["W1blk"].astype(bf).astype(np.float32)
    z = (W1.T @ pay_bf.astype(np.float32)).astype(np.float32)   # [ROWS, T]
    rl = relu(z + W["b1a_rep"]).astype(bf)                      # relu'd bf16
    Ssum = np.zeros((ROWS, S), np.float32)
    for (s0, ioff, bb, nb, d) in lay.runs:
        base = s0 * SUPER + ioff
        seg = rl[:, base:base + nb * d].reshape(ROWS, nb, d).copy()
        ab, chain, tail = fold_schedule(d)
        if ab is not None:
            A, Bw = ab
            seg[:, :, :Bw] = (seg[:, :, :Bw].astype(np.float32)
                              + seg[:, :, A:A + Bw]).astype(bf)
        L = d
        for h in chain:
            seg[:, :, :h] = (seg[:, :, :h].astype(np.float32)
                             + seg[:, :, h:2 * h]).astype(bf)
            L = h
        Ssum[:, bb:bb + nb] = seg[:, :, :L].astype(np.float32).sum(
            axis=2).astype(bf)
    invq = invdeg_rep.astype(bf).astype(np.float32)
    mean_r = (Ssum * invq).astype(bf).astype(np.float32)        # [ROWS, S]
    WCb = W["WCblk"].astype(bf).astype(np.float32)
    NL = W["NLHS"].astype(bf).astype(np.float32)
    rhs = np.concatenate([qn, padinv], axis=0).astype(bf).astype(np.float32)
    p2 = WCb.T @ mean_r + NL.T @ rhs
    hn = relu(p2 + W["bc_rep"]).astype(bf).astype(np.float32)
    W2B = W["W2Bblk"].astype(bf).astype(np.float32)
    out = (W2B.T @ hn).astype(bf)
    return out.astype(np.float32)


# ---------------------------------------------------------------------------
# Bass device program
# ---------------------------------------------------------------------------
def build_bass_program(lay, dtype_np=np.float32, pay_dtype_np=None):
    import concourse.bass as bass
    import concourse.bacc as bacc
    import concourse.tile as tile
    import concourse.mybir as mybir

    dt = mybir.dt.from_np(np.dtype(dtype_np))
    if pay_dtype_np is None:
        pay_dtype_np = dtype_np
    pdt = mybir.dt.from_np(np.dtype(pay_dtype_np))
    f32 = mybir.dt.float32
    T, S = lay.T, lay.S
    AF = mybir.ActivationFunctionType
    ALU = mybir.AluOpType

    nc = bacc.Bacc("TRN2", target_bir_lowering=False, debug=False,
                   num_devices=NCORES)
    NSC = (S + NF - 1) // NF

    # inputs
    pay_d = nc.dram_tensor("payload", [ROWS, T], pdt, kind="ExternalInput")
    w1_d = nc.dram_tensor("W1blk", [ROWS, ROWS], dt, kind="ExternalInput")
    bias3_d = nc.dram_tensor("bias3", [ROWS, 3], f32, kind="ExternalInput")
    # bf16 pack: wc | w2b | nlhs-cols? nlhs is [18, ROWS] separate partitions
    wbf_d = nc.dram_tensor("wbf", [ROWS, 2 * ROWS], dt, kind="ExternalInput")
    fvec_d = nc.dram_tensor("fvec", [ROWS, S], dt, kind="ExternalInput")
    xq_d = nc.dram_tensor("xq", [2 * NGROUPS, S + ROWS], dt,
                          kind="ExternalInput")
    out_d = nc.dram_tensor("out", [ROWS, S], dt, kind="ExternalOutput")

    # group runs by super; split tails at node-chunk boundaries
    runs_by_super = {}
    for (s0, ioff, bb, nb, d) in lay.runs:
        runs_by_super.setdefault(s0, []).append((ioff, bb, nb, d))
    # node output pieces: (k, plo, phi). The last chunk is split so its
    # first half ships before the final super; the final pieces go out on
    # the sync HWDGE queue (payload stream is drained by then) instead of
    # the slow single-engine SWDGE queue.
    pieces = []
    for k in range(NSC):
        w = min(NF, S - k * NF)
        if k < NSC - 1:
            pieces.append((k, 0, w))
        else:
            pieces.append((k, 0, w // 2))
            pieces.append((k, w // 2, w))
    def piece_last_super(k, plo, phi):
        lo = k * NF + plo
        hi = k * NF + phi
        last = 0
        for (s0, ioff, bb, nb, d) in lay.runs:
            if bb < hi and bb + nb > lo:
                last = max(last, s0)
        return last
    node_after = {}
    for (k, plo, phi) in pieces:
        s0 = piece_last_super(k, plo, phi)
        node_after.setdefault(s0, []).append((k, plo, phi))

    n_supers = lay.n_supers
    cps = SUPER // CHUNK
    total_chunks = T // CHUNK
    dve_chunks = set(range(DVE_RELU_EVERY - 1, total_chunks,
                           DVE_RELU_EVERY))

    with tile.TileContext(nc) as tc:
        with (
            tc.tile_pool(name="const", bufs=1) as constp,
            tc.tile_pool(name="persist", bufs=1) as persist,
            tc.tile_pool(name="inp", bufs=4) as inp,
            tc.tile_pool(name="inp0", bufs=1) as inp0,
            tc.tile_pool(name="relu", bufs=4) as relup,
            tc.tile_pool(name="psum", bufs=2,
                         space=bass.MemorySpace.PSUM) as psum,
            tc.tile_pool(name="npsum", bufs=2,
                         space=bass.MemorySpace.PSUM) as npsum,
            tc.tile_pool(name="node", bufs=1) as nodep,
        ):
            # W1 + tiny bias constants first (first matmul/relu need them),
            # on the sync HWDGE queue ahead of the payload stream
            w1 = constp.tile([ROWS, ROWS], dt)
            nc.sync.dma_start(w1[:], w1_d[:])
            bias3 = persist.tile([ROWS, 3], f32)
            nc.sync.dma_start(bias3[:], bias3_d[:])
            warm = persist.tile([ROWS, 1], dt)
            nc.scalar.activation(warm[:], bias3[:, 0:1], AF.Relu)
            # super 0 in two half-super pieces so compute starts after the
            # first 0.39MB lands without paying per-chunk dispatch costs
            NCHUNK_PRE = 1
            HALF = (cps // 2) * CHUNK       # chunk-aligned first piece
            pin0s = {}
            bounds = [(0, HALF), (HALF, min(SUPER, T))]
            for ci, (lo, hi) in enumerate(bounds):
                t = inp0.tile([ROWS, hi - lo], pdt, tag=f"pin0{ci}",
                              name=f"pin0{ci}")
                nc.sync.dma_start(t[:], pay_d[:, lo:hi])
                pin0s[ci] = t
            pre = {}
            # startup prefetch: spread dispatches across idle engine DGE
            # queues (each dispatch costs ~0.8us of queue time; serializing
            # 6 of them on sync delays the ramp)
            pre_eng = {}
            for si in range(NCHUNK_PRE, min(5, n_supers)):
                w = min(SUPER, T - si * SUPER)
                pin = inp.tile([ROWS, w], pdt, tag="pin", name=f"pin_pre{si}")
                pre_eng.get(si, nc.sync).dma_start(
                    pin[:], pay_d[:, si * SUPER:si * SUPER + w])
                pre[si] = pin

            # big aux constants ride the GpSimd SWDGE queue so they overlap
            # the payload HWDGE stream instead of stalling it
            wbf = persist.tile([ROWS, 2 * ROWS], dt)
            nc.gpsimd.dma_start(wbf[:], wbf_d[:])
            xq = persist.tile([2 * NGROUPS, S + ROWS], dt)
            nc.gpsimd.dma_start(xq[:], xq_d[:])
            # invdeg is 700KB: on the SWDGE queue it starves the output
            # pieces for ~30us; ride sync instead (payload stream has 4+
            # supers of headroom before the first node piece needs it)
            fvec = persist.tile([ROWS, S], dt)
            nc.sync.dma_start(fvec[:], fvec_d[:])
            wc = wbf[:, 0:ROWS]
            w2b = wbf[:, ROWS:2 * ROWS]
            invdeg = fvec[:, 0:S]
            b1a = bias3[:, 0:1]
            bc = bias3[:, 1:2]
            b2b = bias3[:, 2:3]
            nrhs = xq[:, 0:S]
            nlhs = xq[:, S:S + ROWS]

            Ssum = [nodep.tile([ROWS, min(NF, S - k * NF)], dt,
                               name=f"Ssum{k}", tag=f"Ssum{k}")
                    for k in range(NSC)]

            def emit_node_chunk(k, plo, phi):
                lo = k * NF + plo
                hi = k * NF + phi
                w = phi - plo
                nm = f"{k}_{plo}"
                final = (k >= NSC - 2)
                mean_r = nodep.tile([ROWS, w], dt, tag=f"mr{nm}",
                                    name=f"mr{nm}")
                with nc.allow_low_precision(reason="bf16 node phase"):
                    nc.vector.tensor_mul(mean_r[:], Ssum[k][:, plo:phi],
                                         invdeg[:, lo:hi])
                p2 = npsum.tile([ROWS, NF], f32, tag="np", name=f"np2_{nm}")
                nc.tensor.matmul(p2[:, :w], wc, mean_r[:],
                                 start=True, stop=False)
                nc.tensor.matmul(p2[:, :w], nlhs, nrhs[:, lo:hi],
                                 start=False, stop=True)
                hn = nodep.tile([ROWS, w], dt, tag=f"hn{nm}", name=f"hn{nm}")
                nc.scalar.activation(hn[:], p2[:, :w], AF.Relu, bias=bc)
                p3 = npsum.tile([ROWS, NF], f32, tag="np", name=f"np3_{nm}")
                nc.tensor.matmul(p3[:, :w], w2b, hn[:],
                                 start=True, stop=True)
                outt = nodep.tile([ROWS, w], dt, tag=f"out{nm}",
                                  name=f"out{nm}")
                with nc.allow_low_precision(reason="bf16 out, bias on host"):
                    nc.scalar.activation(outt[:], p3[:, :w], AF.Identity)
                if final:
                    nc.sync.dma_start(out_d[:, lo:hi], outt[:])
                else:
                    nc.gpsimd.dma_start(out_d[:, lo:hi], outt[:])

            # ---- edge phase ----
            for si in range(n_supers):
                sw = min(SUPER, T - si * SUPER)
                scps = sw // CHUNK
                if si < NCHUNK_PRE:
                    pin = None
                elif si in pre:
                    pin = pre[si]
                else:
                    pin = inp.tile([ROWS, sw], pdt, tag="pin")
                    nc.sync.dma_start(pin[:],
                                      pay_d[:, si * SUPER:si * SUPER + sw])
                rl = relup.tile([ROWS, sw], dt, tag="rl", name=f"rl{si}")
                for cj in range(scps):
                    if si < NCHUNK_PRE:
                        half = 1 if cj * CHUNK >= HALF else 0
                        src = pin0s[half]
                        coff = cj * CHUNK - half * HALF
                    else:
                        src = pin
                        coff = cj * CHUNK
                    ps = psum.tile([ROWS, CHUNK], f32, tag="ps")
                    for m in range(CHUNK // MM_F):
                        nc.tensor.matmul(
                            ps[:, m * MM_F:(m + 1) * MM_F],
                            w1[:],
                            src[:, coff + m * MM_F:coff + (m + 1) * MM_F],
                            start=True, stop=True,
                        )
                    rchunk = rl[:, cj * CHUNK:(cj + 1) * CHUNK]
                    ci = si * cps + cj
                    if ci in dve_chunks:
                        with nc.allow_low_precision(reason="bf16 relu"):
                            nc.vector.tensor_scalar(
                                out=rchunk, in0=ps[:],
                                scalar1=b1a, scalar2=0.0,
                                op0=ALU.add, op1=ALU.max)
                    else:
                        nc.scalar.activation(rchunk, ps[:], AF.Relu,
                                             bias=b1a)
                # compacting ping-pong folds + contiguous tails.
                # scrA/scrB: every fold level writes a DENSE region so the
                # next level (and the tail reduce) reads contiguous data —
                # strided segmented reduce measured 2.1ns/el vs 1.07 dense.
                scrA = relup.tile([ROWS, SUPER // 2], dt, tag="scrA",
                                  name=f"scrA{si}")
                scrB = relup.tile([ROWS, SUPER // 4], dt, tag="scrB",
                                  name=f"scrB{si}")
                runs = runs_by_super.get(si, [])
                sb0 = min(bb for (_, bb, _, _) in runs) if runs else 0
                nbs = sum(nb for (_, _, nb, _) in runs)
                # final fold level of every run lands block-indexed in one
                # common tails region -> ONE merged contiguous reduce per
                # super instead of one (strided) reduce per run
                tails = relup.tile([ROWS, max(8 * nbs, 8)], dt, tag="tails",
                                   name=f"tails{si}")
                for (ioff, bb, nb, d) in runs:
                    ab, chain, tail = fold_schedule(d)
                    assert chain and tail == 8, (d, chain, tail)
                    cur = rl[:, ioff:ioff + nb * d].rearrange(
                        "p (n d) -> p n d", d=d)
                    with nc.allow_low_precision(reason="bf16 fold"):
                        if ab is not None:
                            A, Bw = ab
                            nc.vector.tensor_add(cur[:, :, 0:Bw],
                                                 cur[:, :, 0:Bw],
                                                 cur[:, :, A:A + Bw])
                        for li, h in enumerate(chain):
                            if li == len(chain) - 1:
                                o = (bb - sb0) * 8
                                dst = tails[:, o:o + nb * 8]
                            elif li % 2 == 0:
                                dst = scrA[:, ioff // 2:ioff // 2 + nb * h]
                            else:
                                dst = scrB[:, ioff // 4:ioff // 4 + nb * h]
                            dst3 = dst.rearrange("p (n d) -> p n d", d=h)
                            nc.vector.tensor_add(dst3, cur[:, :, 0:h],
                                                 cur[:, :, h:2 * h])
                            cur = dst3
                # merged tail reduce into Ssum, split at node-chunk bounds
                b = sb0
                left = nbs
                while left > 0:
                    k = b // NF
                    take = min(left, (k + 1) * NF - b)
                    o = (b - sb0) * 8
                    sub = tails[:, o:o + take * 8].rearrange(
                        "p (n d) -> p n d", d=8)
                    with nc.allow_low_precision(reason="bf16 Ssum"):
                        nc.vector.reduce_sum(
                            out=Ssum[k][:, b - k * NF:b - k * NF + take],
                            in_=sub, axis=mybir.AxisListType.X)
                    b += take
                    left -= take
                for (k, plo, phi) in node_after.get(si, []):
                    emit_node_chunk(k, plo, phi)

    nc.compile()
    return nc


# ---------------------------------------------------------------------------
# kernel() entry point
# ---------------------------------------------------------------------------
def _prepare(x, edge_index, edge_attr, weights, dtype_np=np.float32):
    x = np.asarray(x, np.float32)
    edge_attr = np.asarray(edge_attr, np.float32)
    row = np.asarray(edge_index[0], np.int64)
    col = np.asarray(edge_index[1], np.int64)
    lay = build_layout(col)
    src_slots = build_src_slots(lay)
    W = build_weights(*weights)
    q_nodes = ((x * _MINK) * x).sum(axis=1).astype(np.float32)
    attr_mod = edge_attr + q_nodes[row][:, None] * W["alpha"][None, :]
    per_core = []
    for c in range(NCORES):
        pay = build_core_payload(lay, src_slots, attr_mod, c, dtype_np)
        invdeg_rep, qn, padinv = build_core_aux(lay, x, c)
        per_core.append(dict(payload=pay, invdeg=invdeg_rep, qn=qn,
                             padinv=padinv))
    return lay, W, per_core


def _assemble(lay, outs):
    """outs: list of [ROWS, S] per core -> [N, HID]."""
    S = lay.S
    big = np.stack([np.asarray(o, np.float32).reshape(NGROUPS, HID, S)
                    for o in outs])                             # [c,g,j,b]
    arr = big.transpose(3, 0, 1, 2).reshape(S * NLANES, HID)    # (b, c, g)
    res = np.empty((lay.Np, HID), np.float32)
    res[lay.node_sorted] = arr
    return res[:N]


LAST_EXEC_TIME_NS = None
LAST_RESULTS = None


def kernel(x, edge_index, edge_attr, u, batch,
           w1a, b1a, w1b, b1b, w2a, b2a, w2b, b2b):
    global LAST_EXEC_TIME_NS, LAST_RESULTS
    _install_ntff_shim()
    weights = tuple(np.asarray(a, np.float32)
                    for a in (w1a, b1a, w1b, b1b, w2a, b2a, w2b, b2b))
    import ml_dtypes
    dtype_np = np.dtype(ml_dtypes.bfloat16)
    pay_dtype_np = np.dtype(ml_dtypes.float8_e4m3fn)
    lay, W, per_core = _prepare(x, edge_index, edge_attr, weights,
                                pay_dtype_np)

    if os.environ.get("LNB_EMULATE"):
        outs = [emulate_core(lay, pc["payload"], pc["invdeg"], pc["qn"],
                             pc["padinv"], W, dtype_np)
                for pc in per_core]
        return _assemble(lay, outs) + np.asarray(weights[7], np.float32)[None, :]

    from concourse.bass_utils import run_bass_kernel_spmd
    nc = build_bass_program(lay, dtype_np, pay_dtype_np)
    in_maps = []
    S = lay.S
    for pc in per_core:
        wbf = np.concatenate([W["WCblk"], W["W2Bblk"]],
                             axis=1).astype(dtype_np)
        bias3 = np.concatenate([W["b1a_rep"], W["bc_rep"], W["b2b_rep"]],
                               axis=1).astype(np.float32)
        fvec = pc["invdeg"].astype(dtype_np)
        rhs = np.concatenate([pc["qn"], pc["padinv"]], axis=0)
        xq = np.concatenate([rhs, W["NLHS"]], axis=1).astype(dtype_np)
        in_maps.append({
            "payload": pc["payload"], "W1blk": W["W1blk"].astype(dtype_np),
            "wbf": wbf, "bias3": bias3, "fvec": fvec, "xq": xq,
        })
    trace = bool(os.environ.get("BASS_TRACE"))
    res = run_bass_kernel_spmd(nc, in_maps, list(range(NCORES)), trace=trace)
    LAST_EXEC_TIME_NS = res.exec_time_ns
    LAST_RESULTS = res
    outs = [res.results[c]["out"] for c in range(NCORES)]
    return _assemble(lay, outs) + np.asarray(weights[7], np.float32)[None, :]


# revision 53
# speedup vs baseline: 1.0392x; 1.0392x over previous
"""LorentzNodeBlock - Trainium2 Bass kernel, 8 NeuronCores, scatter-free.

Sharding: by DESTINATION node (no collective needed). Host sorts nodes by
in-degree into blocks of 72 (one node per (core, group) lane; 8 cores x 9
groups), pads each block's edge list to a common multiple-of-4 degree, and
packs blocks into 6144-slot "supers" so every lane shares one static layout.

Per-edge pipeline on each core (T ~ 92k slots, 126 partitions = 9 groups x
14 features):
  * q = minkowski(x)[row] is folded into edge_attr on the host
    (attr~ = attr + alpha q, alpha = W_attr^-T w_q, exact) so the payload
    is just 14 fp8-e4m3 rows per group -> 12 MB/core of HBM traffic.
  * PE: block-diagonal first-layer matmul (bf16 weights x fp8 payload)
    into PSUM f32, 512-col matmuls, 3 per 1536-slot chunk.
  * relu(z + b1a): split between ScalarE activation-with-bias (6/7 of
    chunks, 0.83 ns/el) and DVE tensor_scalar add+max (1/7, 1.04 ns/el)
    to balance the two engines; output bf16 into a per-super tile. The
    ACT table set is warmed by a dummy relu during the initial DMA wait.
  * segment-sum: A+B pairwise folds on DVE at the 2x packed rate
    (0.57 ns/el): first fold the excess over the largest power of two A
    (offset A, width d-A, both mult-4 -> 2x engages), then halve A down
    to 8 with compacting ping-pong writes so every level and the final
    8-wide reduce_sum tail read fully contiguous data. Tails of all runs
    land block-indexed in one region -> one merged reduce per super.
    (InstTensorReduce is capped at 1x; in-place strided tails measured
    2.1 ns/el - both are why the fold tree + compaction exist.)
  * pad slots contribute relu(b1a), subtracted exactly in the node phase
    via an extra K-row of the node matmul (rhs row = n_pad * invdeg).
  * node phase (fused second edge-MLP layer + node MLP, WC = w1b@w2a[1:])
    is emitted piecewise right after the super that completes each
    256-node output range; early pieces ship via the GpSimd SWDGE queue,
    the last ones via sync HWDGE (payload stream is drained by then).
  * aux constants ride the SWDGE queue to stay out of the payload FIFO
    (except the 350KB invdeg table - on SWDGE it starves the output
    pieces - which rides sync after the payload prefetches); the first
    super lands as two chunk-aligned pieces, 4 supers prefetched ahead.
    invdeg/Ssum/outputs are bf16 (b2b bias applied on the host).

Measured on trn2 (8 cores, NTFF): 116.3-117.5 us HW exec (runs vary
~+-1us; occasional device thermal throttling adds 10-20us), rel err
4.4e-3 (vs 136.5 us for the v1 baseline). Engine balance at the wall:
ScalarE ~84us, VectorE ~82us busy; span overhead is ~7us framework
preamble, ~6us DMA ramp, ~5us drain, ~19us chunk-semaphore latency.
"""

import os
import sys
import types
import numpy as np

N = 100000
E = 6400000
HID = 14
NCORES = 8
NGROUPS = 9                   # groups per core
NLANES = NCORES * NGROUPS     # 72 lanes; block = NLANES nodes
ROWS = NGROUPS * HID          # 126 payload/psum partitions
CHUNK = 1536                  # edge slots per PSUM chunk (free dim, 3 banks)
SUPER = 6144                  # edge slots per DMA transfer / fold region
MM_F = 512                    # matmul moving free dim (ISA max; 1 PSUM bank)
NF = 256                      # node-chunk output slot width
DPAD = 4                      # block degree padded to multiple of this
                              # (A+B folds only need mult-4 alignment)
TAIL_STOP = 8                 # stop folding when seg len <= this (or odd)

# engine-balance knobs (per-super assignment)
DVE_RELU_EVERY = 7            # every k-th chunk relu runs on DVE (not ACT)
POOL_FOLD_EVERY = 10 ** 9     # gpsimd folds disabled (2.4 ns/elem measured)

_MINK = np.array([-1.0, 1.0, 1.0, 1.0], dtype=np.float32)


# ---------------------------------------------------------------------------
# axon NTFF shim: lets run_bass_kernel_spmd(trace=True) work when the image's
# antenv package lacks axon_hooks. Harmless when tracing is off.
# ---------------------------------------------------------------------------
def _install_ntff_shim():
    try:
        from antenv.axon_hooks import get_axon_ntff_profile_hook  # noqa: F401
        have = True
    except ImportError:
        have = False
    if not have:
        try:
            import antenv
        except ImportError:
            return
        mod = types.ModuleType("antenv.axon_hooks")
        _hook = [None]
        mod.set_axon_ntff_profile_hook = lambda h: _hook.__setitem__(0, h)
        mod.get_axon_ntff_profile_hook = lambda: _hook[0]
        sys.modules["antenv.axon_hooks"] = mod
        antenv.axon_hooks = mod
    try:
        from antenv.axon_hooks import (get_axon_ntff_profile_hook,
                                       set_axon_ntff_profile_hook)
        if get_axon_ntff_profile_hook() is None:
            from trn_agent_boot.trn_boot import _ntff_profile_via_ctypes
            set_axon_ntff_profile_hook(
                _ntff_profile_via_ctypes('/opt/axon/libaxon_pjrt.so'))
    except Exception:
        pass


# ---------------------------------------------------------------------------
# Host-side layout
# ---------------------------------------------------------------------------
class Layout:
    __slots__ = ("node_sorted", "blk_deg", "blk_off", "n_blocks", "S", "T",
                 "runs", "deg", "cs", "edge_sorted", "n_supers", "Np")


def build_layout(col):
    """col: int array [E] of destination nodes. Pure index work.

    Blocks of NLANES nodes sorted by degree; block degree padded to a
    multiple of DPAD. Blocks packed sequentially into SUPER-slot regions
    (a block never crosses a super boundary). Produces `runs`: maximal
    groups of consecutive equal-d blocks within one super:
        (super_idx, slot_off_in_super, block_base, nb, d)
    """
    lay = Layout()
    deg = np.bincount(col, minlength=N).astype(np.int64)
    Np = ((N + NLANES - 1) // NLANES) * NLANES
    n_pad_nodes = Np - N
    deg_p = np.concatenate([deg, np.zeros(n_pad_nodes, np.int64)])
    node_sorted = np.argsort(deg_p, kind="stable")
    B = Np // NLANES
    node_mat = node_sorted.reshape(B, NLANES)
    raw_deg = deg_p[node_mat].max(axis=1)                     # [B]
    blk_deg = ((raw_deg + DPAD - 1) // DPAD) * DPAD           # mult of DPAD
    assert (blk_deg > 0).all()

    # sequential pack into supers
    blk_off = np.zeros(B, np.int64)                            # global slot
    off = 0
    for b in range(B):
        d = int(blk_deg[b])
        rem = SUPER - (off % SUPER)
        if rem < d:
            off += rem                                        # dead tail
        blk_off[b] = off
        off += d
    T = ((off + CHUNK - 1) // CHUNK) * CHUNK
    n_supers = (T + SUPER - 1) // SUPER

    # runs of equal-d consecutive blocks within one super
    runs = []
    b = 0
    while b < B:
        d = int(blk_deg[b])
        s0 = int(blk_off[b]) // SUPER
        nb = 1
        while (b + nb < B and int(blk_deg[b + nb]) == d
               and int(blk_off[b + nb]) // SUPER == s0
               and int(blk_off[b + nb]) == int(blk_off[b]) + nb * d):
            nb += 1
        runs.append((s0, int(blk_off[b]) % SUPER, b, nb, d))
        b += nb

    cs = np.zeros(N + 1, np.int64)
    cs[1:] = np.cumsum(deg)
    lay.node_sorted = node_sorted
    lay.blk_deg = blk_deg
    lay.blk_off = blk_off
    lay.n_blocks = B
    lay.S = B
    lay.T = int(T)
    lay.n_supers = int(n_supers)
    lay.runs = runs
    lay.deg = deg_p
    lay.cs = cs
    lay.edge_sorted = np.argsort(col, kind="stable")
    lay.Np = Np
    return lay


def build_src_slots(lay):
    """[NLANES, T] int64 edge-id per lane slot, -1 for padding."""
    B, T = lay.n_blocks, lay.T
    node_mat = lay.node_sorted.reshape(B, NLANES)
    deg_nm = lay.deg[node_mat]                       # [B, NLANES]
    D = int(lay.blk_deg.max())
    k = np.arange(D, dtype=np.int64)
    valid = k[None, None, :] < deg_nm[:, :, None]    # [B, NLANES, D]
    cs_nm = np.where(node_mat < N, lay.cs[np.minimum(node_mat, N - 1)], 0)
    idx = cs_nm[:, :, None] + k[None, None, :]
    idx = np.minimum(idx, E - 1)
    src = np.where(valid, lay.edge_sorted[idx], -1)  # [B, NLANES, D]
    slot_valid = k[None, :] < lay.blk_deg[:, None]   # [B, D]
    pos = (lay.blk_off[:, None] + k[None, :])        # [B, D]
    out = np.full((NLANES, T), -1, np.int64)
    out[:, pos[slot_valid]] = src.transpose(1, 0, 2)[:, slot_valid]
    return out


def lane_of(c, g):
    return c * NGROUPS + g


def build_core_payload(lay, src_slots, attr_mod, core, dtype=np.float32):
    """[ROWS, T]: rows 14g+j = attr_mod[:, j] of lane (core, g); pad -> 0."""
    T = lay.T
    pay = np.zeros((ROWS, T), dtype)
    for g in range(NGROUPS):
        s = src_slots[lane_of(core, g)]
        m = s >= 0
        sc = np.where(m, s, 0)
        ea = attr_mod[sc].astype(dtype)
        ea[~m] = 0
        pay[HID * g:HID * (g + 1), :] = ea.T
    return pay


def build_core_aux(lay, x, core):
    """invdeg_rep [ROWS,S], qn [NGROUPS,S], padinv [NGROUPS,S]."""
    B = lay.n_blocks
    node_mat = lay.node_sorted.reshape(B, NLANES)
    lanes = [lane_of(core, g) for g in range(NGROUPS)]
    nodes = node_mat[:, lanes].T                          # [NGROUPS, B]
    degs = lay.deg[nodes].astype(np.float32)              # real degree
    invdeg = 1.0 / np.maximum(degs, 1.0)
    invdeg_rep = np.repeat(invdeg, HID, axis=0)           # [ROWS, B]
    npad = lay.blk_deg[None, :].astype(np.float32) - degs  # padded - real
    padinv = npad * invdeg                                 # [NGROUPS, B]
    real = nodes < N
    xn = x[np.minimum(nodes, N - 1)].astype(np.float32)    # [NGROUPS, B, 4]
    xn[~real] = 0
    qn = ((xn * _MINK) * xn).sum(axis=2).astype(np.float32)  # [NGROUPS, B]
    return invdeg_rep, qn, padinv


def build_weights(w1a, b1a, w1b, b1b, w2a, b2a, w2b, b2b):
    def blkdiag(w, nin, nout):
        out = np.zeros((NGROUPS * nin, NGROUPS * nout), np.float32)
        for g in range(NGROUPS):
            out[g * nin:(g + 1) * nin, g * nout:(g + 1) * nout] = w
        return out
    W_attr = w1a[1:]                                       # [14, 14]
    alpha = np.linalg.solve(W_attr.T.astype(np.float64),
                            w1a[0].astype(np.float64)).astype(np.float32)
    relu_b1a = np.maximum(b1a, 0.0)                        # [14]
    WC = w1b @ w2a[1:]                                     # [14, 14]
    W = {}
    W["alpha"] = alpha
    W["W1blk"] = blkdiag(W_attr, HID, HID)                 # [126, 126]
    W["b1a_rep"] = np.tile(b1a, NGROUPS).astype(np.float32)[:, None]
    W["WCblk"] = blkdiag(WC, HID, HID)
    W["bc_rep"] = np.tile(w2a[1:].T @ b1b + b2a,
                          NGROUPS).astype(np.float32)[:, None]
    W["W2Bblk"] = blkdiag(w2b, HID, HID)
    W["b2b_rep"] = np.tile(b2b, NGROUPS).astype(np.float32)[:, None]
    # node-phase lhs [2*NGROUPS, ROWS]: rows 0..G-1 carry w2a[0] (q term),
    # rows G..2G-1 carry -(WC^T relu_b1a) (pad-slot correction)
    corr = -(WC.T @ relu_b1a)                              # [14]
    lhs = np.zeros((2 * NGROUPS, ROWS), np.float32)
    for g in range(NGROUPS):
        lhs[g, g * HID:(g + 1) * HID] = w2a[0]
        lhs[NGROUPS + g, g * HID:(g + 1) * HID] = corr
    W["NLHS"] = lhs
    return W


def fold_schedule(d):
    """A+B fold plan: ((A, B) | None, [h1, h2, ...], tail).

    HW measurement: the DVE 2x packed mode needs the in1 element offset
    and width to be multiples of 4 (offset%4==2 runs 1x, odd offsets hit
    a ~7x slow path). Plan: first fold the excess over the largest
    power of two A (in1 offset = A, width B = d-A, both mult-8), then
    halve A cleanly down to 8 (offsets = powers of two). Every op is 2x.
    """
    A = 1 << (d.bit_length() - 1)
    if A == d:
        ab = None
    else:
        ab = (A, d - A)
    chain = []
    L = A
    while L > TAIL_STOP:
        L //= 2
        chain.append(L)
    return ab, chain, L


# ---------------------------------------------------------------------------
# Numpy emulation of the device program (for layout/logic/precision check)
# ---------------------------------------------------------------------------
def emulate_core(lay, pay_bf, invdeg_rep, qn, padinv, W, dtype_np):
    import ml_dtypes  # noqa: F401
    bf = dtype_np
    T, S = lay.T, lay.S
    relu = lambda v: np.maximum(v, 0.0)
    W1 = W["W1blk"].astype(bf).astype(np.float32)
    z = (W1.T @ pay_bf.astype(np.float32)).astype(np.float32)   # [ROWS, T]
    rl = relu(z + W["b1a_rep"]).astype(bf)                      # relu'd bf16
    Ssum = np.zeros((ROWS, S), np.float32)
    for (s0, ioff, bb, nb, d) in lay.runs:
        base = s0 * SUPER + ioff
        seg = rl[:, base:base + nb * d].reshape(ROWS, nb, d).copy()
        ab, chain, tail = fold_schedule(d)
        if ab is not None:
            A, Bw = ab
            seg[:, :, :Bw] = (seg[:, :, :Bw].astype(np.float32)
                              + seg[:, :, A:A + Bw]).astype(bf)
        L = d
        for h in chain:
            seg[:, :, :h] = (seg[:, :, :h].astype(np.float32)
                             + seg[:, :, h:2 * h]).astype(bf)
            L = h
        Ssum[:, bb:bb + nb] = seg[:, :, :L].astype(np.float32).sum(
            axis=2).astype(bf)
    invq = invdeg_rep.astype(bf).astype(np.float32)
    mean_r = (Ssum * invq).astype(bf).astype(np.float32)        # [ROWS, S]
    WCb = W["WCblk"].astype(bf).astype(np.float32)
    NL = W["NLHS"].astype(bf).astype(np.float32)
    rhs = np.concatenate([qn, padinv], axis=0).astype(bf).astype(np.float32)
    p2 = WCb.T @ mean_r + NL.T @ rhs
    hn = relu(p2 + W["bc_rep"]).astype(bf).astype(np.float32)
    W2B = W["W2Bblk"].astype(bf).astype(np.float32)
    out = (W2B.T @ hn).astype(bf)
    return out.astype(np.float32)


# ---------------------------------------------------------------------------
# Bass device program
# ---------------------------------------------------------------------------
def build_bass_program(lay, dtype_np=np.float32, pay_dtype_np=None):
    import concourse.bass as bass
    import concourse.bacc as bacc
    import concourse.tile as tile
    import concourse.mybir as mybir

    dt = mybir.dt.from_np(np.dtype(dtype_np))
    if pay_dtype_np is None:
        pay_dtype_np = dtype_np
    pdt = mybir.dt.from_np(np.dtype(pay_dtype_np))
    f32 = mybir.dt.float32
    T, S = lay.T, lay.S
    AF = mybir.ActivationFunctionType
    ALU = mybir.AluOpType

    nc = bacc.Bacc("TRN2", target_bir_lowering=False, debug=False,
                   num_devices=NCORES)
    NSC = (S + NF - 1) // NF

    # inputs
    pay_d = nc.dram_tensor("payload", [ROWS, T], pdt, kind="ExternalInput")
    w1_d = nc.dram_tensor("W1blk", [ROWS, ROWS], dt, kind="ExternalInput")
    bias3_d = nc.dram_tensor("bias3", [ROWS, 3], f32, kind="ExternalInput")
    # bf16 pack: wc | w2b | nlhs-cols? nlhs is [18, ROWS] separate partitions
    wbf_d = nc.dram_tensor("wbf", [ROWS, 2 * ROWS], dt, kind="ExternalInput")
    fvec_d = nc.dram_tensor("fvec", [ROWS, S], dt, kind="ExternalInput")
    xq_d = nc.dram_tensor("xq", [2 * NGROUPS, S + ROWS], dt,
                          kind="ExternalInput")
    out_d = nc.dram_tensor("out", [ROWS, S], dt, kind="ExternalOutput")

    # group runs by super; split tails at node-chunk boundaries
    runs_by_super = {}
    for (s0, ioff, bb, nb, d) in lay.runs:
        runs_by_super.setdefault(s0, []).append((ioff, bb, nb, d))
    # node output pieces: (k, plo, phi). The last chunk is split so its
    # first half ships before the final super; the final pieces go out on
    # the sync HWDGE queue (payload stream is drained by then) instead of
    # the slow single-engine SWDGE queue.
    pieces = []
    for k in range(NSC):
        w = min(NF, S - k * NF)
        if k < NSC - 1:
            pieces.append((k, 0, w))
        else:
            pieces.append((k, 0, w // 2))
            pieces.append((k, w // 2, w))
    def piece_last_super(k, plo, phi):
        lo = k * NF + plo
        hi = k * NF + phi
        last = 0
        for (s0, ioff, bb, nb, d) in lay.runs:
            if bb < hi and bb + nb > lo:
                last = max(last, s0)
        return last
    node_after = {}
    for (k, plo, phi) in pieces:
        s0 = piece_last_super(k, plo, phi)
        node_after.setdefault(s0, []).append((k, plo, phi))

    n_supers = lay.n_supers
    cps = SUPER // CHUNK
    total_chunks = T // CHUNK
    dve_chunks = set(range(DVE_RELU_EVERY - 1, total_chunks,
                           DVE_RELU_EVERY))

    with tile.TileContext(nc) as tc:
        with (
            tc.tile_pool(name="const", bufs=1) as constp,
            tc.tile_pool(name="persist", bufs=1) as persist,
            tc.tile_pool(name="inp", bufs=4) as inp,
            tc.tile_pool(name="inp0", bufs=1) as inp0,
            tc.tile_pool(name="relu", bufs=4) as relup,
            tc.tile_pool(name="psum", bufs=2,
                         space=bass.MemorySpace.PSUM) as psum,
            tc.tile_pool(name="npsum", bufs=2,
                         space=bass.MemorySpace.PSUM) as npsum,
            tc.tile_pool(name="node", bufs=1) as nodep,
        ):
            # W1 + tiny bias constants first (first matmul/relu need them),
            # on the sync HWDGE queue ahead of the payload stream
            w1 = constp.tile([ROWS, ROWS], dt)
            nc.sync.dma_start(w1[:], w1_d[:])
            bias3 = persist.tile([ROWS, 3], f32)
            nc.sync.dma_start(bias3[:], bias3_d[:])
            warm = persist.tile([ROWS, 1], dt)
            nc.scalar.activation(warm[:], bias3[:, 0:1], AF.Relu)
            # super 0 in two half-super pieces so compute starts after the
            # first 0.39MB lands without paying per-chunk dispatch costs
            NCHUNK_PRE = 1
            HALF = (cps // 2) * CHUNK       # chunk-aligned first piece
            pin0s = {}
            bounds = [(0, HALF), (HALF, min(SUPER, T))]
            for ci, (lo, hi) in enumerate(bounds):
                t = inp0.tile([ROWS, hi - lo], pdt, tag=f"pin0{ci}",
                              name=f"pin0{ci}")
                nc.sync.dma_start(t[:], pay_d[:, lo:hi])
                pin0s[ci] = t
            pre = {}
            # startup prefetch: spread dispatches across idle engine DGE
            # queues (each dispatch costs ~0.8us of queue time; serializing
            # 6 of them on sync delays the ramp)
            pre_eng = {}
            for si in range(NCHUNK_PRE, min(5, n_supers)):
                w = min(SUPER, T - si * SUPER)
                pin = inp.tile([ROWS, w], pdt, tag="pin", name=f"pin_pre{si}")
                pre_eng.get(si, nc.sync).dma_start(
                    pin[:], pay_d[:, si * SUPER:si * SUPER + w])
                pre[si] = pin

            # big aux constants ride the GpSimd SWDGE queue so they overlap
            # the payload HWDGE stream instead of stalling it
            wbf = persist.tile([ROWS, 2 * ROWS], dt)
            nc.gpsimd.dma_start(wbf[:], wbf_d[:])
            xq = persist.tile([2 * NGROUPS, S + ROWS], dt)
            nc.gpsimd.dma_start(xq[:], xq_d[:])
            # invdeg is 700KB: on the SWDGE queue it starves the output
            # pieces for ~30us; ride sync instead (payload stream has 4+
            # supers of headroom before the first node piece needs it)
            fvec = persist.tile([ROWS, S], dt)
            nc.sync.dma_start(fvec[:], fvec_d[:])
            wc = wbf[:, 0:ROWS]
            w2b = wbf[:, ROWS:2 * ROWS]
            invdeg = fvec[:, 0:S]
            b1a = bias3[:, 0:1]
            bc = bias3[:, 1:2]
            b2b = bias3[:, 2:3]
            nrhs = xq[:, 0:S]
            nlhs = xq[:, S:S + ROWS]

            Ssum = [nodep.tile([ROWS, min(NF, S - k * NF)], dt,
                               name=f"Ssum{k}", tag=f"Ssum{k}")
                    for k in range(NSC)]

            def emit_node_chunk(k, plo, phi):
                lo = k * NF + plo
                hi = k * NF + phi
                w = phi - plo
                nm = f"{k}_{plo}"
                final = (k >= NSC - 2)
                mean_r = nodep.tile([ROWS, w], dt, tag=f"mr{nm}",
                                    name=f"mr{nm}")
                with nc.allow_low_precision(reason="bf16 node phase"):
                    nc.vector.tensor_mul(mean_r[:], Ssum[k][:, plo:phi],
                                         invdeg[:, lo:hi])
                p2 = npsum.tile([ROWS, NF], f32, tag="np", name=f"np2_{nm}")
                nc.tensor.matmul(p2[:, :w], wc, mean_r[:],
                                 start=True, stop=False)
                nc.tensor.matmul(p2[:, :w], nlhs, nrhs[:, lo:hi],
                                 start=False, stop=True)
                hn = nodep.tile([ROWS, w], dt, tag=f"hn{nm}", name=f"hn{nm}")
                nc.scalar.activation(hn[:], p2[:, :w], AF.Relu, bias=bc)
                p3 = npsum.tile([ROWS, NF], f32, tag="np", name=f"np3_{nm}")
                nc.tensor.matmul(p3[:, :w], w2b, hn[:],
                                 start=True, stop=True)
                outt = nodep.tile([ROWS, w], dt, tag=f"out{nm}",
                                  name=f"out{nm}")
                with nc.allow_low_precision(reason="bf16 out, bias on host"):
                    nc.scalar.activation(outt[:], p3[:, :w], AF.Identity)
                if final:
                    nc.sync.dma_start(out_d[:, lo:hi], outt[:])
                else:
                    nc.gpsimd.dma_start(out_d[:, lo:hi], outt[:])

            # ---- edge phase ----
            for si in range(n_supers):
                sw = min(SUPER, T - si * SUPER)
                scps = sw // CHUNK
                if si < NCHUNK_PRE:
                    pin = None
                elif si in pre:
                    pin = pre[si]
                else:
                    pin = inp.tile([ROWS, sw], pdt, tag="pin")
                    nc.sync.dma_start(pin[:],
                                      pay_d[:, si * SUPER:si * SUPER + sw])
                rl = relup.tile([ROWS, sw], dt, tag="rl", name=f"rl{si}")
                for cj in range(scps):
                    if si < NCHUNK_PRE:
                        half = 1 if cj * CHUNK >= HALF else 0
                        src = pin0s[half]
                        coff = cj * CHUNK - half * HALF
                    else:
                        src = pin
                        coff = cj * CHUNK
                    ps = psum.tile([ROWS, CHUNK], f32, tag="ps")
                    for m in range(CHUNK // MM_F):
                        nc.tensor.matmul(
                            ps[:, m * MM_F:(m + 1) * MM_F],
                            w1[:],
                            src[:, coff + m * MM_F:coff + (m + 1) * MM_F],
                            start=True, stop=True,
                        )
                    rchunk = rl[:, cj * CHUNK:(cj + 1) * CHUNK]
                    ci = si * cps + cj
                    if ci in dve_chunks:
                        with nc.allow_low_precision(reason="bf16 relu"):
                            nc.vector.tensor_scalar(
                                out=rchunk, in0=ps[:],
                                scalar1=b1a, scalar2=0.0,
                                op0=ALU.add, op1=ALU.max)
                    else:
                        nc.scalar.activation(rchunk, ps[:], AF.Relu,
                                             bias=b1a)
                # compacting ping-pong folds + contiguous tails.
                # scrA/scrB: every fold level writes a DENSE region so the
                # next level (and the tail reduce) reads contiguous data —
                # strided segmented reduce measured 2.1ns/el vs 1.07 dense.
                scrA = relup.tile([ROWS, SUPER // 2], dt, tag="scrA",
                                  name=f"scrA{si}")
                scrB = relup.tile([ROWS, SUPER // 4], dt, tag="scrB",
                                  name=f"scrB{si}")
                runs = runs_by_super.get(si, [])
                sb0 = min(bb for (_, bb, _, _) in runs) if runs else 0
                nbs = sum(nb for (_, _, nb, _) in runs)
                # final fold level of every run lands block-indexed in one
                # common tails region -> ONE merged contiguous reduce per
                # super instead of one (strided) reduce per run
                tails = relup.tile([ROWS, max(8 * nbs, 8)], dt, tag="tails",
                                   name=f"tails{si}")
                for (ioff, bb, nb, d) in runs:
                    ab, chain, tail = fold_schedule(d)
                    assert chain and tail == 8, (d, chain, tail)
                    cur = rl[:, ioff:ioff + nb * d].rearrange(
                        "p (n d) -> p n d", d=d)
                    with nc.allow_low_precision(reason="bf16 fold"):
                        if ab is not None:
                            A, Bw = ab
                            nc.vector.tensor_add(cur[:, :, 0:Bw],
                                                 cur[:, :, 0:Bw],
                                                 cur[:, :, A:A + Bw])
                        for li, h in enumerate(chain):
                            if li == len(chain) - 1:
                                o = (bb - sb0) * 8
                                dst = tails[:, o:o + nb * 8]
                            elif li % 2 == 0:
                                dst = scrA[:, ioff // 2:ioff // 2 + nb * h]
                            else:
                                dst = scrB[:, ioff // 4:ioff // 4 + nb * h]
                            dst3 = dst.rearrange("p (n d) -> p n d", d=h)
                            nc.vector.tensor_add(dst3, cur[:, :, 0:h],
                                                 cur[:, :, h:2 * h])
                            cur = dst3
                # merged tail reduce into Ssum, split at node-chunk bounds
                b = sb0
                left = nbs
                while left > 0:
                    k = b // NF
                    take = min(left, (k + 1) * NF - b)
                    o = (b - sb0) * 8
                    sub = tails[:, o:o + take * 8].rearrange(
                        "p (n d) -> p n d", d=8)
                    with nc.allow_low_precision(reason="bf16 Ssum"):
                        nc.vector.reduce_sum(
                            out=Ssum[k][:, b - k * NF:b - k * NF + take],
                            in_=sub, axis=mybir.AxisListType.X)
                    b += take
                    left -= take
                for (k, plo, phi) in node_after.get(si, []):
                    emit_node_chunk(k, plo, phi)

    nc.compile()
    return nc


# ---------------------------------------------------------------------------
# kernel() entry point
# ---------------------------------------------------------------------------
def _prepare(x, edge_index, edge_attr, weights, dtype_np=np.float32):
    x = np.asarray(x, np.float32)
    edge_attr = np.asarray(edge_attr, np.float32)
    row = np.asarray(edge_index[0], np.int64)
    col = np.asarray(edge_index[1], np.int64)
    lay = build_layout(col)
    src_slots = build_src_slots(lay)
    W = build_weights(*weights)
    q_nodes = ((x * _MINK) * x).sum(axis=1).astype(np.float32)
    attr_mod = edge_attr + q_nodes[row][:, None] * W["alpha"][None, :]
    per_core = []
    for c in range(NCORES):
        pay = build_core_payload(lay, src_slots, attr_mod, c, dtype_np)
        invdeg_rep, qn, padinv = build_core_aux(lay, x, c)
        per_core.append(dict(payload=pay, invdeg=invdeg_rep, qn=qn,
                             padinv=padinv))
    return lay, W, per_core


def _assemble(lay, outs):
    """outs: list of [ROWS, S] per core -> [N, HID]."""
    S = lay.S
    big = np.stack([np.asarray(o, np.float32).reshape(NGROUPS, HID, S)
                    for o in outs])                             # [c,g,j,b]
    arr = big.transpose(3, 0, 1, 2).reshape(S * NLANES, HID)    # (b, c, g)
    res = np.empty((lay.Np, HID), np.float32)
    res[lay.node_sorted] = arr
    return res[:N]


LAST_EXEC_TIME_NS = None
LAST_RESULTS = None


def kernel(x, edge_index, edge_attr, u, batch,
           w1a, b1a, w1b, b1b, w2a, b2a, w2b, b2b):
    global LAST_EXEC_TIME_NS, LAST_RESULTS
    _install_ntff_shim()
    weights = tuple(np.asarray(a, np.float32)
                    for a in (w1a, b1a, w1b, b1b, w2a, b2a, w2b, b2b))
    import ml_dtypes
    dtype_np = np.dtype(ml_dtypes.bfloat16)
    pay_dtype_np = np.dtype(ml_dtypes.float8_e4m3fn)
    lay, W, per_core = _prepare(x, edge_index, edge_attr, weights,
                                pay_dtype_np)

    if os.environ.get("LNB_EMULATE"):
        outs = [emulate_core(lay, pc["payload"], pc["invdeg"], pc["qn"],
                             pc["padinv"], W, dtype_np)
                for pc in per_core]
        return _assemble(lay, outs) + np.asarray(weights[7], np.float32)[None, :]

    from concourse.bass_utils import run_bass_kernel_spmd
    nc = build_bass_program(lay, dtype_np, pay_dtype_np)
    in_maps = []
    S = lay.S
    for pc in per_core:
        wbf = np.concatenate([W["WCblk"], W["W2Bblk"]],
                             axis=1).astype(dtype_np)
        bias3 = np.concatenate([W["b1a_rep"], W["bc_rep"], W["b2b_rep"]],
                               axis=1).astype(np.float32)
        fvec = pc["invdeg"].astype(dtype_np)
        rhs = np.concatenate([pc["qn"], pc["padinv"]], axis=0)
        xq = np.concatenate([rhs, W["NLHS"]], axis=1).astype(dtype_np)
        in_maps.append({
            "payload": pc["payload"], "W1blk": W["W1blk"].astype(dtype_np),
            "wbf": wbf, "bias3": bias3, "fvec": fvec, "xq": xq,
        })
    trace = bool(os.environ.get("BASS_TRACE"))
    res = run_bass_kernel_spmd(nc, in_maps, list(range(NCORES)), trace=trace)
    LAST_EXEC_TIME_NS = res.exec_time_ns
    LAST_RESULTS = res
    outs = [res.results[c]["out"] for c in range(NCORES)]
    return _assemble(lay, outs) + np.asarray(weights[7], np.float32)[None, :]
